# revision 27
# baseline (speedup 1.0000x reference)
"""Trainium2 Bass kernel for nn_LogicLayer.

Math: out[b,o] = sum_f softmax(weights[o])[f] * op_f(a,b),
      a = x[b, idx0[o]], b = x[b, idx1[o]].
All 16 logic ops are affine in {1, a, b, ab}, so
      out[b,o] = C0[o] + CA[o]*a + CB[o]*b + CAB[o]*a*b
with per-neuron coefficients Cj[o] = sum_f probs[o,f] * T[f,j].

Strategy (8 NeuronCores, out_dim sharded 8192 neurons/core):
 - Host: transpose x -> x_T [IN_DIM, B] in fp16 so a gathered "column of x"
   is a contiguous 512B row; split into two 32768-row halves (dma_gather
   uses int16 indices, max 32768 rows).
 - Per core, bucket its 8192 columns by (half(idx0), half(idx1)) so each
   dma_gather call reads one half with int16 indices; pad buckets to a
   multiple of 128 with index 0 (valid row; padded outputs dropped on host).
 - Device: SWDGE dma_gather rows of x_T into SBUF [128, slots, 256] fp16.
   Calls are 384 indices (25 descriptors/engine: small enough to double-
   buffer in the per-queue descriptor ring; >=1152 idx hangs the ring) and
   rotate across all 4 SWDGE queues: each queue runs on its own Q7 core
   pair with its own descriptor ring, so descriptor emission parallelizes
   and per-queue completion waits overlap. A tiny warmup gather absorbs
   the ~10us Q7 library IRAM load while the index DMAs land.
 - Softmax -> affine coefficients in f32 on Scalar/Vector engines, then
   per slot [128 positions x 256 batch]: ACT computes t1 = CAB*a + CB,
   t2 = CA*a + C0 alternates between ACT and DVE tensor_scalar (balances
   the two engines' per-instruction overheads), and chunk-wide DVE does
   out = t1*b + t2. Output written back as fp16, upcast on host.
 - Host: invert the bucket permutation and transpose back to [B, OUT_DIM].
"""

import os

import numpy as np

B = 256
IN_DIM = 65536
OUT_DIM = 65536
NFN = 16
NCORES = 8
SHARD = OUT_DIM // NCORES
HALF = IN_DIM // 2
P = 128

# Coefficient table: op_f(a,b) = T[f,0] + T[f,1]*a + T[f,2]*b + T[f,3]*ab
_T = np.array(
    [
        [0, 0, 0, 0],    # false
        [0, 0, 0, 1],    # a AND b
        [0, 1, 0, -1],   # a AND NOT b
        [0, 1, 0, 0],    # a
        [0, 0, 1, -1],   # NOT a AND b
        [0, 0, 1, 0],    # b
        [0, 1, 1, -2],   # XOR
        [0, 1, 1, -1],   # OR
        [1, -1, -1, 1],  # NOR
        [1, -1, -1, 2],  # XNOR
        [1, 0, -1, 0],   # NOT b
        [1, 0, -1, 1],   # a OR NOT b
        [1, -1, 0, 0],   # NOT a
        [1, -1, 0, 1],   # NOT a OR b
        [1, 0, 0, -1],   # NAND
        [1, 0, 0, 0],    # true
    ],
    dtype=np.float32,
)

_BUILD_CACHE = {}
LAST_RESULTS = None  # BassKernelResults of the most recent run (for profiling)


def _wrap_idx(idx16):
    """[n] int16 -> [128, n//16] wrapped: position i at (i%16, i//16),
    replicated across the 8 groups of 16 partitions (one per Q7 core)."""
    w = idx16.reshape(-1, 16).T  # [16, n/16]
    return np.ascontiguousarray(np.tile(w, (8, 1)))


def _build_kernel(caps):
    """Build + compile the SPMD program for bucket capacities `caps` (4-tuple,
    each a multiple of 128). Returns (nc, npad)."""
    key = tuple(caps)
    if key in _BUILD_CACHE:
        return _BUILD_CACHE[key]

    import concourse.bacc as bacc
    import concourse.mybir as mybir
    import concourse.tile as tile
    from concourse import library_config

    npad = int(sum(caps))
    nslot = npad // P
    offs = np.concatenate([[0], np.cumsum(caps)]).astype(int)

    nc = bacc.Bacc(
        "TRN2",
        target_bir_lowering=False,
        debug=False,
        dynamic_dma_scratch_size=int(os.environ.get("K_DMA_SCRATCH", "16384")),
        num_swdge_queues=4,
    )
    f32 = mybir.dt.float32
    f16 = mybir.dt.float16
    i16 = mybir.dt.int16

    xA_d = nc.dram_tensor("xA", [HALF, B], f16, kind="ExternalInput")
    xB_d = nc.dram_tensor("xB", [HALF, B], f16, kind="ExternalInput")
    # combined index array (see kernel() for layout): buckets 0/3 hold
    # 128-blocks of a-idx and b-idx interleaved (merged a+b gather calls);
    # buckets 1/2 hold [ia_k ; ib_k] back to back.
    ix_d = nc.dram_tensor("ix", [P, 2 * npad // 16], i16, kind="ExternalInput")
    # w pre-wrapped on host to [P, nslot, NFN] so the load is contiguous
    w_d = nc.dram_tensor("w", [P, nslot * NFN], f32, kind="ExternalInput")
    out_d = nc.dram_tensor("out", [P, nslot * B], f16, kind="ExternalOutput")

    Exp = mybir.ActivationFunctionType.Exp
    Ident = mybir.ActivationFunctionType.Identity
    X = mybir.AxisListType.X
    Mult = mybir.AluOpType.mult
    Add = mybir.AluOpType.add

    MAX_CALL = int(os.environ.get("K_MAX_CALL", "384"))
    SP = os.environ.get("K_SINGLE_PACKET", "1") == "1"

    from contextlib import ExitStack

    with tile.TileContext(nc) as tc, ExitStack() as ctx:
        nc.gpsimd.load_library(library_config.mlp)
        consts = ctx.enter_context(tc.tile_pool(name="consts", bufs=1))
        work = ctx.enter_context(
            tc.tile_pool(name="work", bufs=int(os.environ.get("K_BUFS", "4")))
        )

        # --- load index lists (stay resident) ---
        ix_t = consts.tile([P, 2 * npad // 16], i16)
        nc.sync.dma_start(out=ix_t[:], in_=ix_d[:])

        # --- warmup: one tiny gather absorbs the Q7 library IRAM load while
        # the index DMAs land ---
        warm_i = consts.tile([P, 8], i16)
        nc.vector.memset(warm_i[:], 0)
        warm_o = consts.tile([P, 1, B], f16)
        nc.gpsimd.dma_gather(
            out_ap=warm_o[:],
            in_ap=xA_d[:],
            idxs_ap=warm_i[:],
            num_idxs=P,
            num_idxs_reg=P,
            elem_size=B,
            single_packet=True,
            queue_num=0,
        )

        # --- softmax -> affine coefficients for all positions ---
        w_t = consts.tile([P, nslot * NFN], f32)
        nc.sync.dma_start(out=w_t[:], in_=w_d[:])
        e_t = consts.tile([P, nslot * NFN], f32)
        nc.scalar.activation(e_t[:], w_t[:], Exp)
        e3 = e_t[:].rearrange("p (s f) -> p s f", f=NFN)

        def rsum(dst, src_ap):
            nc.vector.tensor_reduce(dst, src_ap, axis=X, op=mybir.AluOpType.add)

        s_t = consts.tile([P, nslot], f32)     # sum_f e
        rden = consts.tile([P, nslot], f32)    # 1/sum
        c0_t = consts.tile([P, nslot], f32)
        ca_t = consts.tile([P, nslot], f32)
        cb_t = consts.tile([P, nslot], f32)
        cab_t = consts.tile([P, nslot], f32)
        tmp1 = consts.tile([P, nslot], f32)
        tmp2 = consts.tile([P, nslot], f32)

        rsum(s_t[:], e3)
        nc.vector.reciprocal(out=rden[:], in_=s_t[:])

        # C0: +{8..15}
        rsum(c0_t[:], e3[:, :, 8:16])
        # CA: +{2,3} +{6,7} -{8,9} -{12,13}
        rsum(ca_t[:], e3[:, :, 2:4])
        rsum(tmp1[:], e3[:, :, 6:8])
        nc.vector.tensor_add(ca_t[:], ca_t[:], tmp1[:])
        rsum(tmp1[:], e3[:, :, 8:10])
        nc.vector.tensor_sub(ca_t[:], ca_t[:], tmp1[:])
        rsum(tmp1[:], e3[:, :, 12:14])
        nc.vector.tensor_sub(ca_t[:], ca_t[:], tmp1[:])
        # CB: +{4..7} -{8..11}
        rsum(cb_t[:], e3[:, :, 4:8])
        rsum(tmp1[:], e3[:, :, 8:12])
        nc.vector.tensor_sub(cb_t[:], cb_t[:], tmp1[:])
        # CAB: +e1 -e2 -e4 -2*e6 -e7 +e8 +2*e9 +e11 +e13 -e14
        #    = (e1+e8+e11+e13) - (e2+e4+e7+e14) + 2*(e9-e6)
        def ef(f):
            return e3[:, :, f]

        nc.vector.tensor_add(cab_t[:], ef(1), ef(8))
        nc.vector.tensor_add(cab_t[:], cab_t[:], ef(11))
        nc.vector.tensor_add(cab_t[:], cab_t[:], ef(13))
        nc.vector.tensor_add(tmp1[:], ef(2), ef(4))
        nc.vector.tensor_add(tmp1[:], tmp1[:], ef(7))
        nc.vector.tensor_add(tmp1[:], tmp1[:], ef(14))
        nc.vector.tensor_sub(cab_t[:], cab_t[:], tmp1[:])
        nc.vector.tensor_sub(tmp2[:], ef(9), ef(6))
        nc.vector.tensor_add(cab_t[:], cab_t[:], tmp2[:])
        nc.vector.tensor_add(cab_t[:], cab_t[:], tmp2[:])
        # normalize
        for ct in (c0_t, ca_t, cb_t, cab_t):
            nc.vector.tensor_mul(ct[:], ct[:], rden[:])

        # --- main loop over bucket-aligned chunks of columns ---
        # each bucket is split into equal-sized chunks (multiples of 128) so
        # every gather call is the same size: no tiny remainder calls that pay
        # full completion-latency periods. Buckets 0/3 (a and b read the same
        # half of x) use ONE merged gather per chunk with a/b index blocks
        # interleaved; buckets 1/2 use separate a and b calls.
        merged_on = os.environ.get("K_MERGED", "0") == "1"
        chunk_merged = int(os.environ.get("K_CHUNK_MERGED", "512"))
        chunk_mixed = int(os.environ.get("K_CHUNK_POS", "1152"))
        chunks = []  # (bucket, p0, p1)
        tail_sz = int(os.environ.get("K_TAIL", "0"))
        for k in range(4):
            cap = int(caps[k])
            if cap == 0:
                continue
            # carve a small final chunk off the last bucket so the kernel
            # tail (last gather -> compute -> writeback) is short
            tail = tail_sz if k == 3 and cap >= tail_sz + P else 0
            cap -= tail
            chunk_cap = chunk_merged if (merged_on and k in (0, 3)) else chunk_mixed
            nch = max(1, -(-cap // chunk_cap))
            lo = int(offs[k])
            for i in range(nch):
                sz = P * (
                    (cap * (i + 1)) // (nch * P) - (cap * i) // (nch * P)
                )
                chunks.append((k, lo, lo + sz))
                lo += sz
            if tail:
                chunks.append((k, lo, lo + tail))
                lo += tail
            assert lo == offs[k + 1]
        qrot = 0
        for (k, p0g, p1g) in chunks:
            cbase, cs = p0g // P, (p1g - p0g) // P
            src_a = xA_d if k < 2 else xB_d
            src_b = xA_d if k % 2 == 0 else xB_d
            t1_t = work.tile([P, cs, B], f16)
            t2_t = work.tile([P, cs, B], f16)
            o_t = work.tile([P, cs * B], f16)
            if merged_on and k in (0, 3):
                # merged: one call gathers a and b, 128-blocks interleaved
                g_t = work.tile([P, 2 * cs, B], f16)
                i0 = 2 * offs[k] + 2 * (p0g - offs[k])
                nc.gpsimd.dma_gather(
                    out_ap=g_t[:],
                    in_ap=src_a[:],
                    idxs_ap=ix_t[:, i0 // 16 : (i0 + 2 * (p1g - p0g)) // 16],
                    num_idxs=2 * (p1g - p0g),
                    num_idxs_reg=2 * (p1g - p0g),
                    elem_size=B,
                    single_packet=SP,
                    queue_num=qrot % 4,
                )
                qrot += 1
                a_t3 = g_t[:, 0 : 2 * cs : 2, :]
                b_t3 = g_t[:, 1 : 2 * cs : 2, :]
            else:
                a_t = work.tile([P, cs, B], f16)
                b_t = work.tile([P, cs, B], f16)
                ia0 = 2 * offs[k] + (p0g - offs[k])
                ib0 = 2 * offs[k] + int(caps[k]) + (p0g - offs[k])
                n_tot = p1g - p0g
                # all a-calls first: the per-slot t1/t2 compute depends only
                # on a, so it can start while the b-calls are still draining
                for (dst, src, i0) in ((a_t, src_a, ia0), (b_t, src_b, ib0)):
                    lo = 0
                    while lo < n_tot:
                        n = min(MAX_CALL, n_tot - lo)
                        sl, sh = lo // P, (lo + n) // P
                        nc.gpsimd.dma_gather(
                            out_ap=dst[:, sl:sh, :],
                            in_ap=src[:],
                            idxs_ap=ix_t[
                                :, (i0 + lo) // 16 : (i0 + lo + n) // 16
                            ],
                            num_idxs=n,
                            num_idxs_reg=n,
                            elem_size=B,
                            single_packet=SP,
                            queue_num=qrot % 4,
                        )
                        qrot += 1
                        lo += n
                a_t3 = a_t[:]
                b_t3 = b_t[:]
            o3 = o_t[:].rearrange("p (s e) -> p s e", e=B)
            for s in range(cs):
                g = cbase + s
                # t1 = CAB*a + CB   (ScalarE, per-partition scale/bias)
                nc.scalar.activation(
                    t1_t[:, s, :], a_t3[:, s, :], Ident,
                    bias=cb_t[:, g : g + 1], scale=cab_t[:, g : g + 1],
                )
                # t2 = CA*a + C0, alternating between ScalarE and VectorE to
                # balance the two engines' per-slot instruction overheads
                if s % 2 == 0:
                    nc.vector.tensor_scalar(
                        t2_t[:, s, :], a_t3[:, s, :],
                        ca_t[:, g : g + 1], c0_t[:, g : g + 1],
                        Mult, Add,
                    )
                else:
                    nc.scalar.activation(
                        t2_t[:, s, :], a_t3[:, s, :], Ident,
                        bias=c0_t[:, g : g + 1], scale=ca_t[:, g : g + 1],
                    )
            # out = t1*b + t2   (VectorE, chunk-wide)
            nc.vector.tensor_mul(o3[:], t1_t[:], b_t3)
            nc.vector.tensor_add(o3[:], o3[:], t2_t[:])
            nc.sync.dma_start(
                out=out_d[:, cbase * B : (cbase + cs) * B], in_=o_t[:]
            )

    nc.compile()
    _BUILD_CACHE[key] = (nc, npad)
    return nc, npad


def kernel(x, weights, indices):
    from concourse.bass_utils import run_bass_kernel_spmd

    x = np.asarray(x, dtype=np.float32)
    weights = np.asarray(weights, dtype=np.float32)
    indices = np.asarray(indices, dtype=np.int64)

    x_T = np.ascontiguousarray(x.T.astype(np.float16))  # [IN_DIM, B] fp16
    xA = x_T[:HALF]
    xB = x_T[HALF:]

    # --- per-core bucketing ---
    percore = []
    counts_all = np.zeros((NCORES, 4), dtype=np.int64)
    for c in range(NCORES):
        sl = slice(c * SHARD, (c + 1) * SHARD)
        i0 = indices[0, sl]
        i1 = indices[1, sl]
        bid = (i0 >= HALF).astype(np.int64) * 2 + (i1 >= HALF).astype(np.int64)
        order = np.argsort(bid, kind="stable")
        counts = np.bincount(bid, minlength=4)
        counts_all[c] = counts
        percore.append((sl, i0, i1, bid, order, counts))

    caps = tuple(
        int(-(-int(counts_all[:, k].max()) // P) * P) for k in range(4)
    )
    nc, npad = _build_kernel(caps)
    nslot = npad // P
    offs = np.concatenate([[0], np.cumsum(caps)]).astype(int)

    in_maps = []
    pos_maps = []  # per core: global column index per position (-1 = pad)
    for c in range(NCORES):
        sl, i0, i1, bid, order, counts = percore[c]
        ia = np.zeros(npad, dtype=np.int16)
        ib = np.zeros(npad, dtype=np.int16)
        pos = np.full(npad, -1, dtype=np.int64)
        w_pad = np.zeros((npad, NFN), dtype=np.float32)
        w_shard = weights[sl]
        for k in range(4):
            selk = order[np.searchsorted(bid[order], k) :][: counts[k]]
            o, n = int(offs[k]), int(counts[k])
            ia[o : o + n] = (i0[selk] - (HALF if k >= 2 else 0)).astype(np.int16)
            ib[o : o + n] = (i1[selk] - (HALF if k % 2 else 0)).astype(np.int16)
            pos[o : o + n] = sl.start + selk
            w_pad[o : o + n] = w_shard[selk]
        # combined index array: buckets 0/3 interleave a/b in 128-blocks
        # (merged single-call gathers); buckets 1/2 store [ia_k ; ib_k].
        ix = np.zeros(2 * npad, dtype=np.int16)
        for k in range(4):
            o, cap = int(offs[k]), int(caps[k])
            ia_k = ia[o : o + cap]
            ib_k = ib[o : o + cap]
            if os.environ.get("K_MERGED", "0") == "1" and k in (0, 3):
                ix[2 * o : 2 * o + 2 * cap] = np.stack(
                    [ia_k.reshape(-1, P), ib_k.reshape(-1, P)], axis=1
                ).reshape(-1)
            else:
                ix[2 * o : 2 * o + cap] = ia_k
                ix[2 * o + cap : 2 * o + 2 * cap] = ib_k
        # wrap to [P, nslot*NFN]: position i = s*128 + p -> partition p, slot s
        w_wrapped = np.ascontiguousarray(
            w_pad.reshape(nslot, P, NFN).transpose(1, 0, 2)
        ).reshape(P, nslot * NFN)
        in_maps.append(
            {
                "xA": xA,
                "xB": xB,
                "ix": _wrap_idx(ix),
                "w": w_wrapped,
            }
        )
        pos_maps.append(pos)

    res = run_bass_kernel_spmd(nc, in_maps, core_ids=list(range(NCORES)))
    global LAST_RESULTS
    LAST_RESULTS = res

    out = np.empty((B, OUT_DIM), dtype=np.float32)
    for c in range(NCORES):
        o = res.results[c]["out"].reshape(P, nslot, B).astype(np.float32)
        rows = np.ascontiguousarray(o.transpose(1, 0, 2)).reshape(npad, B)
        pos = pos_maps[c]
        valid = pos >= 0
        out[:, pos[valid]] = rows[valid].T
    return out


# revision 28
# speedup vs baseline: 1.0320x; 1.0320x over previous
"""Trainium2 Bass kernel for nn_LogicLayer.

Math: out[b,o] = sum_f softmax(weights[o])[f] * op_f(a,b),
      a = x[b, idx0[o]], b = x[b, idx1[o]].
All 16 logic ops are affine in {1, a, b, ab}, so
      out[b,o] = C0[o] + CA[o]*a + CB[o]*b + CAB[o]*a*b
with per-neuron coefficients Cj[o] = sum_f probs[o,f] * T[f,j].

Strategy (8 NeuronCores, out_dim sharded 8192 neurons/core):
 - Host: transpose x -> x_T [IN_DIM, B] in fp16 so a gathered "column of x"
   is a contiguous 512B row; split into two 32768-row halves (dma_gather
   uses int16 indices, max 32768 rows).
 - Per core, bucket its 8192 columns by (half(idx0), half(idx1)) so each
   dma_gather call reads one half with int16 indices; pad buckets to a
   multiple of 128 with index 0 (valid row; padded outputs dropped on host).
 - Device: SWDGE dma_gather rows of x_T into SBUF [128, slots, 256] fp16.
   Calls are 384 indices (25 descriptors/engine: small enough to double-
   buffer in the per-queue descriptor ring; >=1152 idx hangs the ring) and
   rotate across all 4 SWDGE queues: each queue runs on its own Q7 core
   pair with its own descriptor ring, so descriptor emission parallelizes
   and per-queue completion waits overlap. A tiny warmup gather absorbs
   the ~10us Q7 library IRAM load while the index DMAs land.
 - Softmax -> affine coefficients in f32 on Scalar/Vector engines, then
   per slot [128 positions x 256 batch]: ACT computes t1 = CAB*a + CB,
   t2 = CA*a + C0 alternates between ACT and DVE tensor_scalar (balances
   the two engines' per-instruction overheads), and chunk-wide DVE does
   out = t1*b + t2. Output written back as fp16, upcast on host.
 - Host: invert the bucket permutation and transpose back to [B, OUT_DIM].
"""

import os

import numpy as np

B = 256
IN_DIM = 65536
OUT_DIM = 65536
NFN = 16
NCORES = 8
SHARD = OUT_DIM // NCORES
HALF = IN_DIM // 2
P = 128

# Coefficient table: op_f(a,b) = T[f,0] + T[f,1]*a + T[f,2]*b + T[f,3]*ab
_T = np.array(
    [
        [0, 0, 0, 0],    # false
        [0, 0, 0, 1],    # a AND b
        [0, 1, 0, -1],   # a AND NOT b
        [0, 1, 0, 0],    # a
        [0, 0, 1, -1],   # NOT a AND b
        [0, 0, 1, 0],    # b
        [0, 1, 1, -2],   # XOR
        [0, 1, 1, -1],   # OR
        [1, -1, -1, 1],  # NOR
        [1, -1, -1, 2],  # XNOR
        [1, 0, -1, 0],   # NOT b
        [1, 0, -1, 1],   # a OR NOT b
        [1, -1, 0, 0],   # NOT a
        [1, -1, 0, 1],   # NOT a OR b
        [1, 0, 0, -1],   # NAND
        [1, 0, 0, 0],    # true
    ],
    dtype=np.float32,
)

_BUILD_CACHE = {}
LAST_RESULTS = None  # BassKernelResults of the most recent run (for profiling)


def _wrap_idx(idx16):
    """[n] int16 -> [128, n//16] wrapped: position i at (i%16, i//16),
    replicated across the 8 groups of 16 partitions (one per Q7 core)."""
    w = idx16.reshape(-1, 16).T  # [16, n/16]
    return np.ascontiguousarray(np.tile(w, (8, 1)))


def _build_kernel(caps):
    """Build + compile the SPMD program for bucket capacities `caps` (4-tuple,
    each a multiple of 128). Returns (nc, npad)."""
    key = tuple(caps)
    if key in _BUILD_CACHE:
        return _BUILD_CACHE[key]

    import concourse.bacc as bacc
    import concourse.mybir as mybir
    import concourse.tile as tile
    from concourse import library_config

    npad = int(sum(caps))
    nslot = npad // P
    offs = np.concatenate([[0], np.cumsum(caps)]).astype(int)

    nc = bacc.Bacc(
        "TRN2",
        target_bir_lowering=False,
        debug=False,
        dynamic_dma_scratch_size=int(os.environ.get("K_DMA_SCRATCH", "16384")),
        num_swdge_queues=4,
    )
    f32 = mybir.dt.float32
    f16 = mybir.dt.float16
    i16 = mybir.dt.int16

    xA_d = nc.dram_tensor("xA", [HALF, B], f16, kind="ExternalInput")
    xB_d = nc.dram_tensor("xB", [HALF, B], f16, kind="ExternalInput")
    # combined index array (see kernel() for layout): buckets 0/3 hold
    # 128-blocks of a-idx and b-idx interleaved (merged a+b gather calls);
    # buckets 1/2 hold [ia_k ; ib_k] back to back.
    ix_d = nc.dram_tensor("ix", [P, 2 * npad // 16], i16, kind="ExternalInput")
    # w pre-wrapped on host to [P, nslot, NFN] so the load is contiguous
    w_d = nc.dram_tensor("w", [P, nslot * NFN], f32, kind="ExternalInput")
    out_d = nc.dram_tensor("out", [P, nslot * B], f16, kind="ExternalOutput")

    Exp = mybir.ActivationFunctionType.Exp
    Ident = mybir.ActivationFunctionType.Identity
    X = mybir.AxisListType.X
    Mult = mybir.AluOpType.mult
    Add = mybir.AluOpType.add

    MAX_CALL = int(os.environ.get("K_MAX_CALL", "384"))
    SP = os.environ.get("K_SINGLE_PACKET", "1") == "1"

    from contextlib import ExitStack

    with tile.TileContext(nc) as tc, ExitStack() as ctx:
        nc.gpsimd.load_library(library_config.mlp)
        consts = ctx.enter_context(tc.tile_pool(name="consts", bufs=1))
        work = ctx.enter_context(
            tc.tile_pool(name="work", bufs=int(os.environ.get("K_BUFS", "8")))
        )

        # --- load index lists (stay resident) ---
        ix_t = consts.tile([P, 2 * npad // 16], i16)
        nc.sync.dma_start(out=ix_t[:], in_=ix_d[:])

        # --- warmup: one tiny gather absorbs the Q7 library IRAM load while
        # the index DMAs land ---
        warm_i = consts.tile([P, 8], i16)
        nc.vector.memset(warm_i[:], 0)
        warm_o = consts.tile([P, 1, B], f16)
        nc.gpsimd.dma_gather(
            out_ap=warm_o[:],
            in_ap=xA_d[:],
            idxs_ap=warm_i[:],
            num_idxs=P,
            num_idxs_reg=P,
            elem_size=B,
            single_packet=True,
            queue_num=0,
        )

        # --- softmax -> affine coefficients for all positions ---
        w_t = consts.tile([P, nslot * NFN], f32)
        nc.sync.dma_start(out=w_t[:], in_=w_d[:])
        e_t = consts.tile([P, nslot * NFN], f32)
        nc.scalar.activation(e_t[:], w_t[:], Exp)
        e3 = e_t[:].rearrange("p (s f) -> p s f", f=NFN)

        def rsum(dst, src_ap):
            nc.vector.tensor_reduce(dst, src_ap, axis=X, op=mybir.AluOpType.add)

        s_t = consts.tile([P, nslot], f32)     # sum_f e
        rden = consts.tile([P, nslot], f32)    # 1/sum
        c0_t = consts.tile([P, nslot], f32)
        ca_t = consts.tile([P, nslot], f32)
        cb_t = consts.tile([P, nslot], f32)
        cab_t = consts.tile([P, nslot], f32)
        tmp1 = consts.tile([P, nslot], f32)
        tmp2 = consts.tile([P, nslot], f32)

        rsum(s_t[:], e3)
        nc.vector.reciprocal(out=rden[:], in_=s_t[:])

        # C0: +{8..15}
        rsum(c0_t[:], e3[:, :, 8:16])
        # CA: +{2,3} +{6,7} -{8,9} -{12,13}
        rsum(ca_t[:], e3[:, :, 2:4])
        rsum(tmp1[:], e3[:, :, 6:8])
        nc.vector.tensor_add(ca_t[:], ca_t[:], tmp1[:])
        rsum(tmp1[:], e3[:, :, 8:10])
        nc.vector.tensor_sub(ca_t[:], ca_t[:], tmp1[:])
        rsum(tmp1[:], e3[:, :, 12:14])
        nc.vector.tensor_sub(ca_t[:], ca_t[:], tmp1[:])
        # CB: +{4..7} -{8..11}
        rsum(cb_t[:], e3[:, :, 4:8])
        rsum(tmp1[:], e3[:, :, 8:12])
        nc.vector.tensor_sub(cb_t[:], cb_t[:], tmp1[:])
        # CAB: +e1 -e2 -e4 -2*e6 -e7 +e8 +2*e9 +e11 +e13 -e14
        #    = (e1+e8+e11+e13) - (e2+e4+e7+e14) + 2*(e9-e6)
        def ef(f):
            return e3[:, :, f]

        nc.vector.tensor_add(cab_t[:], ef(1), ef(8))
        nc.vector.tensor_add(cab_t[:], cab_t[:], ef(11))
        nc.vector.tensor_add(cab_t[:], cab_t[:], ef(13))
        nc.vector.tensor_add(tmp1[:], ef(2), ef(4))
        nc.vector.tensor_add(tmp1[:], tmp1[:], ef(7))
        nc.vector.tensor_add(tmp1[:], tmp1[:], ef(14))
        nc.vector.tensor_sub(cab_t[:], cab_t[:], tmp1[:])
        nc.vector.tensor_sub(tmp2[:], ef(9), ef(6))
        nc.vector.tensor_add(cab_t[:], cab_t[:], tmp2[:])
        nc.vector.tensor_add(cab_t[:], cab_t[:], tmp2[:])
        # normalize
        for ct in (c0_t, ca_t, cb_t, cab_t):
            nc.vector.tensor_mul(ct[:], ct[:], rden[:])

        # --- main loop over bucket-aligned chunks of columns ---
        # each bucket is split into equal-sized chunks (multiples of 128) so
        # every gather call is the same size: no tiny remainder calls that pay
        # full completion-latency periods. Buckets 0/3 (a and b read the same
        # half of x) use ONE merged gather per chunk with a/b index blocks
        # interleaved; buckets 1/2 use separate a and b calls.
        merged_on = os.environ.get("K_MERGED", "0") == "1"
        chunk_merged = int(os.environ.get("K_CHUNK_MERGED", "512"))
        chunk_mixed = int(os.environ.get("K_CHUNK_POS", "1024"))
        chunks = []  # (bucket, p0, p1)
        tail_sz = int(os.environ.get("K_TAIL", "0"))
        for k in range(4):
            cap = int(caps[k])
            if cap == 0:
                continue
            # carve a small final chunk off the last bucket so the kernel
            # tail (last gather -> compute -> writeback) is short
            tail = tail_sz if k == 3 and cap >= tail_sz + P else 0
            cap -= tail
            chunk_cap = chunk_merged if (merged_on and k in (0, 3)) else chunk_mixed
            nch = max(1, -(-cap // chunk_cap))
            lo = int(offs[k])
            for i in range(nch):
                sz = P * (
                    (cap * (i + 1)) // (nch * P) - (cap * i) // (nch * P)
                )
                chunks.append((k, lo, lo + sz))
                lo += sz
            if tail:
                chunks.append((k, lo, lo + tail))
                lo += tail
            assert lo == offs[k + 1]
        qrot = 0
        for (k, p0g, p1g) in chunks:
            cbase, cs = p0g // P, (p1g - p0g) // P
            src_a = xA_d if k < 2 else xB_d
            src_b = xA_d if k % 2 == 0 else xB_d
            t1_t = work.tile([P, cs, B], f16)
            t2_t = work.tile([P, cs, B], f16)
            o_t = work.tile([P, cs * B], f16)
            if merged_on and k in (0, 3):
                # merged: one call gathers a and b, 128-blocks interleaved
                g_t = work.tile([P, 2 * cs, B], f16)
                i0 = 2 * offs[k] + 2 * (p0g - offs[k])
                nc.gpsimd.dma_gather(
                    out_ap=g_t[:],
                    in_ap=src_a[:],
                    idxs_ap=ix_t[:, i0 // 16 : (i0 + 2 * (p1g - p0g)) // 16],
                    num_idxs=2 * (p1g - p0g),
                    num_idxs_reg=2 * (p1g - p0g),
                    elem_size=B,
                    single_packet=SP,
                    queue_num=qrot % 4,
                )
                qrot += 1
                a_t3 = g_t[:, 0 : 2 * cs : 2, :]
                b_t3 = g_t[:, 1 : 2 * cs : 2, :]
            else:
                a_t = work.tile([P, cs, B], f16)
                b_t = work.tile([P, cs, B], f16)
                ia0 = 2 * offs[k] + (p0g - offs[k])
                ib0 = 2 * offs[k] + int(caps[k]) + (p0g - offs[k])
                n_tot = p1g - p0g
                # all a-calls first: the per-slot t1/t2 compute depends only
                # on a, so it can start while the b-calls are still draining
                for (dst, src, i0) in ((a_t, src_a, ia0), (b_t, src_b, ib0)):
                    lo = 0
                    while lo < n_tot:
                        n = min(MAX_CALL, n_tot - lo)
                        sl, sh = lo // P, (lo + n) // P
                        nc.gpsimd.dma_gather(
                            out_ap=dst[:, sl:sh, :],
                            in_ap=src[:],
                            idxs_ap=ix_t[
                                :, (i0 + lo) // 16 : (i0 + lo + n) // 16
                            ],
                            num_idxs=n,
                            num_idxs_reg=n,
                            elem_size=B,
                            single_packet=SP,
                            queue_num=qrot % 4,
                        )
                        qrot += 1
                        lo += n
                a_t3 = a_t[:]
                b_t3 = b_t[:]
            o3 = o_t[:].rearrange("p (s e) -> p s e", e=B)
            for s in range(cs):
                g = cbase + s
                # t1 = CAB*a + CB   (ScalarE, per-partition scale/bias)
                nc.scalar.activation(
                    t1_t[:, s, :], a_t3[:, s, :], Ident,
                    bias=cb_t[:, g : g + 1], scale=cab_t[:, g : g + 1],
                )
                # t2 = CA*a + C0, alternating between ScalarE and VectorE to
                # balance the two engines' per-slot instruction overheads
                if s % 2 == 0:
                    nc.vector.tensor_scalar(
                        t2_t[:, s, :], a_t3[:, s, :],
                        ca_t[:, g : g + 1], c0_t[:, g : g + 1],
                        Mult, Add,
                    )
                else:
                    nc.scalar.activation(
                        t2_t[:, s, :], a_t3[:, s, :], Ident,
                        bias=c0_t[:, g : g + 1], scale=ca_t[:, g : g + 1],
                    )
            # out = t1*b + t2   (VectorE, chunk-wide)
            nc.vector.tensor_mul(o3[:], t1_t[:], b_t3)
            nc.vector.tensor_add(o3[:], o3[:], t2_t[:])
            nc.sync.dma_start(
                out=out_d[:, cbase * B : (cbase + cs) * B], in_=o_t[:]
            )

    nc.compile()
    _BUILD_CACHE[key] = (nc, npad)
    return nc, npad


def kernel(x, weights, indices):
    from concourse.bass_utils import run_bass_kernel_spmd

    x = np.asarray(x, dtype=np.float32)
    weights = np.asarray(weights, dtype=np.float32)
    indices = np.asarray(indices, dtype=np.int64)

    x_T = np.ascontiguousarray(x.T.astype(np.float16))  # [IN_DIM, B] fp16
    xA = x_T[:HALF]
    xB = x_T[HALF:]

    # --- per-core bucketing ---
    percore = []
    counts_all = np.zeros((NCORES, 4), dtype=np.int64)
    for c in range(NCORES):
        sl = slice(c * SHARD, (c + 1) * SHARD)
        i0 = indices[0, sl]
        i1 = indices[1, sl]
        bid = (i0 >= HALF).astype(np.int64) * 2 + (i1 >= HALF).astype(np.int64)
        order = np.argsort(bid, kind="stable")
        counts = np.bincount(bid, minlength=4)
        counts_all[c] = counts
        percore.append((sl, i0, i1, bid, order, counts))

    caps = tuple(
        int(-(-int(counts_all[:, k].max()) // P) * P) for k in range(4)
    )
    nc, npad = _build_kernel(caps)
    nslot = npad // P
    offs = np.concatenate([[0], np.cumsum(caps)]).astype(int)

    in_maps = []
    pos_maps = []  # per core: global column index per position (-1 = pad)
    for c in range(NCORES):
        sl, i0, i1, bid, order, counts = percore[c]
        ia = np.zeros(npad, dtype=np.int16)
        ib = np.zeros(npad, dtype=np.int16)
        pos = np.full(npad, -1, dtype=np.int64)
        w_pad = np.zeros((npad, NFN), dtype=np.float32)
        w_shard = weights[sl]
        for k in range(4):
            selk = order[np.searchsorted(bid[order], k) :][: counts[k]]
            o, n = int(offs[k]), int(counts[k])
            ia[o : o + n] = (i0[selk] - (HALF if k >= 2 else 0)).astype(np.int16)
            ib[o : o + n] = (i1[selk] - (HALF if k % 2 else 0)).astype(np.int16)
            pos[o : o + n] = sl.start + selk
            w_pad[o : o + n] = w_shard[selk]
        # combined index array: buckets 0/3 interleave a/b in 128-blocks
        # (merged single-call gathers); buckets 1/2 store [ia_k ; ib_k].
        ix = np.zeros(2 * npad, dtype=np.int16)
        for k in range(4):
            o, cap = int(offs[k]), int(caps[k])
            ia_k = ia[o : o + cap]
            ib_k = ib[o : o + cap]
            if os.environ.get("K_MERGED", "0") == "1" and k in (0, 3):
                ix[2 * o : 2 * o + 2 * cap] = np.stack(
                    [ia_k.reshape(-1, P), ib_k.reshape(-1, P)], axis=1
                ).reshape(-1)
            else:
                ix[2 * o : 2 * o + cap] = ia_k
                ix[2 * o + cap : 2 * o + 2 * cap] = ib_k
        # wrap to [P, nslot*NFN]: position i = s*128 + p -> partition p, slot s
        w_wrapped = np.ascontiguousarray(
            w_pad.reshape(nslot, P, NFN).transpose(1, 0, 2)
        ).reshape(P, nslot * NFN)
        in_maps.append(
            {
                "xA": xA,
                "xB": xB,
                "ix": _wrap_idx(ix),
                "w": w_wrapped,
            }
        )
        pos_maps.append(pos)

    res = run_bass_kernel_spmd(nc, in_maps, core_ids=list(range(NCORES)))
    global LAST_RESULTS
    LAST_RESULTS = res

    out = np.empty((B, OUT_DIM), dtype=np.float32)
    for c in range(NCORES):
        o = res.results[c]["out"].reshape(P, nslot, B).astype(np.float32)
        rows = np.ascontiguousarray(o.transpose(1, 0, 2)).reshape(npad, B)
        pos = pos_maps[c]
        valid = pos >= 0
        out[:, pos[valid]] = rows[valid].T
    return out
